# revision 43
# baseline (speedup 1.0000x reference)
"""Trainium2 Bass kernel for nn_Attention_3032246911698 (sparse_attention).

Computes, per batch row b:
    score_dec = v[0] @ W_v.T + attn_b                      # [B, H]
    score_enc = einsum('ble,he->blh', encoder_out, W_e)    # [B, L, H]
    en        = tanh(score_dec[:,None,:] + score_enc)      # [B, L, H]
    att       = einsum('blh,h->bl', en, v_w[0])            # [B, L]
    att       = where(mask == 0, -1e10, att)
    out       = softmax(att, axis=1)                       # [B, L]

Sharding: data-parallel over batch B=16 across 8 NeuronCores (2 rows each).
Weights are replicated.  No cross-core communication is needed.

Device dataflow per core (Bc=2, L=2048, H=1024, E=2H=2048):
  - host pre-transposes/casts the small replicated weights:
      attn_wT  [3072, 1024] bf16  (rows 0:1024 = W_v.T, rows 1024:3072 = W_e.T)
      decT     [1024, Bc]   bf16, attn_b [1024,1] f32, v_w [1024,1] bf16
  - score_dec computed on-device with 64 tiny matmuls.
  - main loop: stream encoder_out in [128 tok, 2048 e] f32 tiles, cast to
    bf16 (ACT), transpose 128x128 blocks on the TensorEngine into PSUM,
    copy to SBUF (DVE) forming encT tiles [e=128, t=512]; then 16
    accumulating bf16 matmuls per h-chunk produce score[h=128, t=512] in
    PSUM; tanh+bias (ACT, bias = score_dec column) writes en bf16; 8 more
    matmuls against v_w reduce over h into att[1, t=512]; mask+softmax on
    DVE/ACT; store [Bc, 2048] f32.
"""

import os
import sys

import numpy as np

for _p in ("/opt/trn_rl_repo", "/root/.axon_site/_ro/trn_rl_repo"):
    if os.path.isdir(_p) and _p not in sys.path:
        sys.path.append(_p)

import concourse.bass as bass
import concourse.mybir as mybir
import concourse.tile as tile
from concourse import bacc
from concourse.bass_utils import run_bass_kernel_spmd
from concourse.masks import make_identity

try:
    import ml_dtypes

    BF16 = ml_dtypes.bfloat16
except ImportError:  # jax always ships ml_dtypes, but be safe
    import jax.numpy as jnp

    BF16 = jnp.bfloat16

F32 = mybir.dt.float32
BF = mybir.dt.bfloat16

N_CORES = 8
B, L, H = 16, 2048, 1024
E = 2 * H
BC = B // N_CORES          # 2 batch rows per core
TCH = 512                  # tokens per t-chunk
NSUB = TCH // 128          # 128-token subtiles per chunk
NCHUNK = L // TCH          # t-chunks per batch row
NEG_INF = -1.0e10


def build_nc():
    # Bacc (not raw Bass): its compile pipeline legalizes multi-wait sync via
    # event semaphores — walrus only accepts one sync-wait per instruction.
    nc = bacc.Bacc(num_swdge_queues=4)

    enc = nc.declare_dram_parameter("encoder_out", [BC, L, E], F32, isOutput=False)
    # (mask-1)*1e10 precast to bf16: 0 where kept, ~-1e10 where masked; added
    # into the attention PSUM via a K=1 matmul so no tensor-tensor op needed.
    maskadd = nc.declare_dram_parameter("maskadd", [BC, L], BF, isOutput=False)
    wT = nc.declare_dram_parameter("attn_wT", [3 * H, H], BF, isOutput=False)
    decT = nc.declare_dram_parameter("decT", [H, BC], BF, isOutput=False)
    bcol = nc.declare_dram_parameter("attn_bT", [H, 1], F32, isOutput=False)
    vwcol = nc.declare_dram_parameter("v_wT", [H, 1], BF, isOutput=False)
    out = nc.declare_dram_parameter("out", [BC, L], F32, isOutput=True)

    KC = H // 128            # 8 h/hi chunks
    EC = E // 128            # 16 e chunks

    with tile.TileContext(nc) as tc:
        with (
            tc.tile_pool(name="consts", bufs=1) as consts,
            tc.tile_pool(name="scratch", bufs=6, space="DRAM") as scratch_pool,
            tc.tile_pool(name="encT", bufs=4) as encT_pool,
            tc.tile_pool(name="en", bufs=2) as en_pool,
            tc.tile_pool(name="rowbig", bufs=2) as rowbig_pool,
            tc.tile_pool(name="rowsmall", bufs=1) as rowsmall_pool,
            tc.tile_pool(name="psum_score", bufs=4, space="PSUM") as score_psum,
            tc.tile_pool(name="psum_att", bufs=2, space="PSUM") as att_psum,
        ):
            # ---- constants / weights ---------------------------------------
            ones1 = consts.tile([1, 1], BF)
            nc.gpsimd.memset(ones1, 1.0)

            # W_v half only — W_e streams in per-ec pieces AFTER the first
            # chunk's transpose so the 4MB load doesn't head-of-line-block
            # the sync HWDGE queue at startup.
            w_tile = consts.tile([128, 3 * KC, H], BF)      # [p, chunk, h_out]
            wTr = wT.rearrange("(c p) h -> p c h", p=128)
            nc.sync.dma_start(w_tile[:, 0:KC, :], wTr[:, 0:KC, :])

            dec_tile = consts.tile([128, KC, BC], BF)
            nc.sync.dma_start(dec_tile, decT.rearrange("(c p) b -> p c b", p=128))

            b_tile = consts.tile([128, KC, 1], F32)
            nc.sync.dma_start(b_tile, bcol.rearrange("(c p) o -> p c o", p=128))

            vw_tile = consts.tile([128, KC, 1], BF)
            nc.sync.dma_start(vw_tile, vwcol.rearrange("(c p) o -> p c o", p=128))

            # ---- score_dec = dec @ W_v.T + attn_b, stored transposed -------
            # sd_tile[:, hoc, b] = sum_hi W_vT[hi, ho] * dec[hi, b] + attn_b[ho]
            sd_tile = consts.tile([128, KC, BC], F32)
            for hoc in range(KC):
                ps_sd = att_psum.tile([128, BC], F32, tag="attps")
                for hic in range(KC):
                    nc.tensor.matmul(
                        ps_sd,
                        lhsT=w_tile[:, hic, hoc * 128:(hoc + 1) * 128],
                        rhs=dec_tile[:, hic, :],
                        start=(hic == 0),
                        stop=(hic == KC - 1),
                    )
                # ACT (not DVE tensor_scalar): the TensorScalarPtr ISA struct
                # only carries one sync-wait slot and this op needs two.
                nc.scalar.activation(
                    sd_tile[:, hoc, :],
                    ps_sd,
                    mybir.ActivationFunctionType.Identity,
                    bias=b_tile[:, hoc, :],
                )

            # Both mask rows up-front on SWDGE, ahead of the cast traffic and
            # off the sync HWDGE queue (a copy between transposes would cost
            # an xbar mode transition there).
            maskbs = []
            for b in range(BC):
                mb = rowsmall_pool.tile([1, L], BF, tag=f"maskb{b}")
                nc.gpsimd.dma_start(mb, maskadd[b:b + 1, :])
                maskbs.append(mb)

            # Hoisted feed for batch 0 chunk 0 (128 tokens): cast + transpose
            # commit BEFORE the big W_e load so the PE has data within ~10us.
            HEAD0 = 128
            encT0 = encT_pool.tile([128, EC, HEAD0], BF, tag="encT")
            scratch0 = scratch_pool.tile([HEAD0, E], BF, tag="scratch")
            for ts in range(HEAD0 // 64):
                nc.gpsimd.dma_start(
                    scratch0[ts * 64:(ts + 1) * 64, :],
                    enc[0, ts * 64:(ts + 1) * 64, :],
                )
            nc.sync.dma_start(encT0[:, :, :], scratch0[:, :], transpose=True)

            # Now stream W_e in 16 per-ec pieces; score matmuls only need
            # piece ec before their (hc, ec) step, so compute starts while
            # later pieces are still in flight.
            for ec in range(2 * KC):
                nc.sync.dma_start(
                    w_tile[:, KC + ec, :], wTr[:, KC + ec, :]
                )

            # ---- main loop --------------------------------------------------
            # Batch 0 starts with small chunks: the SWDGE cast feed needs
            # ~2us/128 tokens, so a 128-token head chunk lets the PE start
            # ~25us earlier than waiting for a full 512-token chunk.
            head = [(0, 128), (128, 128), (256, 256), (512, 512),
                    (1024, 512), (1536, 512)]
            full = [(tch * TCH, TCH) for tch in range(NCHUNK)]
            for b in range(BC):
                chunks = head if b == 0 else full
                logits = rowbig_pool.tile([1, L], F32, tag="logits")
                mchunk = rowbig_pool.tile([1, len(head)], F32, tag="mchunk")
                maskb = maskbs[b]
                for ci, (t0, tw) in enumerate(chunks):
                    if b == 0 and ci == 0:
                        encT = encT0   # hoisted above
                    else:
                        encT = encT_pool.tile([128, EC, tw], BF, tag="encT")
                        # SWDGE cast-DMA DRAM->DRAM (f32 -> bf16 scratch),
                        # split across the SWDGE queues, then ONE DRAM->SBUF
                        # xbar transpose per chunk: [tw tok, 2048 e] lands as
                        # encT[e % 128, e // 128, t] = enc[t, e].
                        scratch = scratch_pool.tile([tw, E], BF, tag="scratch")
                        for ts in range(max(tw // 64, 1)):
                            nc.gpsimd.dma_start(
                                scratch[ts * 64:(ts + 1) * 64, :],
                                enc[b, t0 + ts * 64:t0 + (ts + 1) * 64, :],
                            )
                        nc.sync.dma_start(
                            encT[:, :, :], scratch[:, :], transpose=True
                        )

                    en_big = en_pool.tile([128, KC, tw], BF, tag="en_big")
                    for hc in range(KC):
                        ps_score = score_psum.tile([128, tw], F32, tag="ps_score")
                        for ec in range(EC):
                            nc.tensor.matmul(
                                ps_score,
                                lhsT=w_tile[:, KC + ec, hc * 128:(hc + 1) * 128],
                                rhs=encT[:, ec, :],
                                start=(ec == 0),
                                stop=(ec == EC - 1),
                            )
                        nc.scalar.activation(
                            en_big[:, hc, :],
                            ps_score,
                            mybir.ActivationFunctionType.Tanh,
                            bias=sd_tile[:, hc, b:b + 1],
                        )

                    ps_att = att_psum.tile([1, tw], F32, tag="attps")
                    for hc in range(KC):
                        nc.tensor.matmul(
                            ps_att,
                            lhsT=vw_tile[:, hc, :],
                            rhs=en_big[:, hc, :],
                            start=(hc == 0),
                            stop=False,
                        )
                    # += (mask-1)*1e10 as a K=1 rank-1 update: masked tokens
                    # drop to ~-1e10 with no elementwise mask op anywhere.
                    nc.tensor.matmul(
                        ps_att,
                        lhsT=ones1,
                        rhs=maskb[:, t0:t0 + tw],
                        start=False,
                        stop=True,
                    )
                    nc.vector.tensor_copy(logits[:, t0:t0 + tw], ps_att)
                    # per-chunk max, computed while the PE crunches on — the
                    # final softmax then only reduces a handful of values.
                    nc.vector.reduce_max(
                        mchunk[:, ci:ci + 1],
                        logits[:, t0:t0 + tw],
                        axis=mybir.AxisListType.X,
                    )

                # ---- softmax over L on a single partition row --------------
                mx = rowsmall_pool.tile([1, 1], F32, tag="mx")
                nc.vector.reduce_max(
                    mx, mchunk[:, 0:len(chunks)], axis=mybir.AxisListType.X
                )
                negmx = rowsmall_pool.tile([1, 1], F32, tag="negmx")
                nc.scalar.mul(negmx, mx, -1.0)
                exps = rowsmall_pool.tile([1, L], F32, tag="exps")
                sumx = rowsmall_pool.tile([1, 1], F32, tag="sumx")
                nc.scalar.activation(
                    exps,
                    logits,
                    mybir.ActivationFunctionType.Exp,
                    bias=negmx[:, :],
                    accum_out=sumx,
                )
                rcp = rowsmall_pool.tile([1, 1], F32, tag="rcp")
                nc.vector.reciprocal(rcp, sumx)
                orow = rowbig_pool.tile([1, L], F32, tag="orow")
                nc.vector.tensor_scalar_mul(orow, exps, rcp[:, :])
                nc.gpsimd.dma_start(out[b:b + 1, :], orow)

    nc.finalize()
    return nc


_NC_CACHE = None


def _get_nc():
    global _NC_CACHE
    if _NC_CACHE is None:
        _NC_CACHE = build_nc()
    return _NC_CACHE


def prepare_in_maps(encoder_out, mask, v, attn_w, attn_b, v_w):
    encoder_out = np.ascontiguousarray(np.asarray(encoder_out, dtype=np.float32))
    maskadd = ((np.asarray(mask, dtype=np.float32) - 1.0) * 1.0e10).astype(BF16)
    wTb = np.ascontiguousarray(np.asarray(attn_w, dtype=np.float32).T).astype(BF16)
    decTb = np.ascontiguousarray(np.asarray(v[0], dtype=np.float32).T).astype(BF16)
    bcol = np.ascontiguousarray(np.asarray(attn_b, dtype=np.float32).reshape(H, 1))
    vwcol = np.ascontiguousarray(
        np.asarray(v_w, dtype=np.float32).reshape(H, 1)
    ).astype(BF16)

    in_maps = []
    for c in range(N_CORES):
        s = slice(c * BC, (c + 1) * BC)
        in_maps.append(
            {
                "encoder_out": encoder_out[s],
                "maskadd": maskadd[s],
                "attn_wT": wTb,
                "decT": np.ascontiguousarray(decTb[:, s]),
                "attn_bT": bcol,
                "v_wT": vwcol,
            }
        )
    return in_maps


def run(inputs, trace=False):
    nc = _get_nc()
    in_maps = prepare_in_maps(**inputs)
    res = run_bass_kernel_spmd(nc, in_maps, core_ids=list(range(N_CORES)), trace=trace)
    out = np.concatenate([res.results[c]["out"] for c in range(N_CORES)], axis=0)
    return out.astype(np.float32), res


def kernel(**inputs):
    out, _ = run(inputs, trace=False)
    return out


# revision 47
# speedup vs baseline: 1.1701x; 1.1701x over previous
"""Trainium2 Bass kernel for nn_Attention_3032246911698 (sparse_attention).

Computes, per batch row b:
    score_dec = v[0] @ W_v.T + attn_b                      # [B, H]
    score_enc = einsum('ble,he->blh', encoder_out, W_e)    # [B, L, H]
    en        = tanh(score_dec[:,None,:] + score_enc)      # [B, L, H]
    att       = einsum('blh,h->bl', en, v_w[0])            # [B, L]
    att       = where(mask == 0, -1e10, att)
    out       = softmax(att, axis=1)                       # [B, L]

Sharding: data-parallel over batch B=16 across 8 NeuronCores (2 rows each).
Weights are replicated.  No cross-core communication is needed.

Device dataflow per core (Bc=2, L=2048, H=1024, E=2H=2048):
  - host pre-transposes/casts the small replicated weights:
      attn_wT  [3072, 1024] bf16  (rows 0:1024 = W_v.T, rows 1024:3072 = W_e.T)
      decT     [1024, Bc]   bf16, attn_b [1024,1] f32, v_w [1024,1] bf16
  - score_dec computed on-device with 64 tiny matmuls.
  - main loop: stream encoder_out in [128 tok, 2048 e] f32 tiles, cast to
    bf16 (ACT), transpose 128x128 blocks on the TensorEngine into PSUM,
    copy to SBUF (DVE) forming encT tiles [e=128, t=512]; then 16
    accumulating bf16 matmuls per h-chunk produce score[h=128, t=512] in
    PSUM; tanh+bias (ACT, bias = score_dec column) writes en bf16; 8 more
    matmuls against v_w reduce over h into att[1, t=512]; mask+softmax on
    DVE/ACT; store [Bc, 2048] f32.
"""

import os
import sys

import numpy as np

for _p in ("/opt/trn_rl_repo", "/root/.axon_site/_ro/trn_rl_repo"):
    if os.path.isdir(_p) and _p not in sys.path:
        sys.path.append(_p)

import concourse.bass as bass
import concourse.mybir as mybir
import concourse.tile as tile
from concourse import bacc
from concourse.bass_utils import run_bass_kernel_spmd
from concourse.masks import make_identity

try:
    import ml_dtypes

    BF16 = ml_dtypes.bfloat16
except ImportError:  # jax always ships ml_dtypes, but be safe
    import jax.numpy as jnp

    BF16 = jnp.bfloat16

F32 = mybir.dt.float32
BF = mybir.dt.bfloat16

N_CORES = 8
B, L, H = 16, 2048, 1024
E = 2 * H
BC = B // N_CORES          # 2 batch rows per core
TCH = 512                  # tokens per t-chunk
NSUB = TCH // 128          # 128-token subtiles per chunk
NCHUNK = L // TCH          # t-chunks per batch row
NEG_INF = -1.0e10


def build_nc():
    # Bacc (not raw Bass): its compile pipeline legalizes multi-wait sync via
    # event semaphores — walrus only accepts one sync-wait per instruction.
    nc = bacc.Bacc(num_swdge_queues=4)

    enc = nc.declare_dram_parameter("encoder_out", [BC, L, E], F32, isOutput=False)
    # (mask-1)*1e10 precast to bf16: 0 where kept, ~-1e10 where masked; added
    # into the attention PSUM via a K=1 matmul so no tensor-tensor op needed.
    maskadd = nc.declare_dram_parameter("maskadd", [BC, L], BF, isOutput=False)
    wT = nc.declare_dram_parameter("attn_wT", [3 * H, H], BF, isOutput=False)
    decT = nc.declare_dram_parameter("decT", [H, BC], BF, isOutput=False)
    bcol = nc.declare_dram_parameter("attn_bT", [H, 1], F32, isOutput=False)
    vwcol = nc.declare_dram_parameter("v_wT", [H, 1], BF, isOutput=False)
    out = nc.declare_dram_parameter("out", [BC, L], F32, isOutput=True)

    KC = H // 128            # 8 h/hi chunks
    EC = E // 128            # 16 e chunks

    with tile.TileContext(nc) as tc:
        with (
            tc.tile_pool(name="consts", bufs=1) as consts,
            tc.tile_pool(name="scratch", bufs=6, space="DRAM") as scratch_pool,
            tc.tile_pool(name="encT", bufs=4) as encT_pool,
            tc.tile_pool(name="en", bufs=2) as en_pool,
            tc.tile_pool(name="rowbig", bufs=2) as rowbig_pool,
            tc.tile_pool(name="rowsmall", bufs=1) as rowsmall_pool,
            tc.tile_pool(name="psum_score", bufs=4, space="PSUM") as score_psum,
            tc.tile_pool(name="psum_att", bufs=2, space="PSUM") as att_psum,
        ):
            # ---- constants / weights ---------------------------------------
            ones1 = consts.tile([1, 1], BF)
            nc.gpsimd.memset(ones1, 1.0)

            w_tile = consts.tile([128, 3 * KC, H], BF)      # [p, chunk, h_out]
            nc.sync.dma_start(w_tile, wT.rearrange("(c p) h -> p c h", p=128))

            dec_tile = consts.tile([128, KC, BC], BF)
            nc.sync.dma_start(dec_tile, decT.rearrange("(c p) b -> p c b", p=128))

            b_tile = consts.tile([128, KC, 1], F32)
            nc.sync.dma_start(b_tile, bcol.rearrange("(c p) o -> p c o", p=128))

            vw_tile = consts.tile([128, KC, 1], BF)
            nc.sync.dma_start(vw_tile, vwcol.rearrange("(c p) o -> p c o", p=128))

            # ---- score_dec = dec @ W_v.T + attn_b, stored transposed -------
            # sd_tile[:, hoc, b] = sum_hi W_vT[hi, ho] * dec[hi, b] + attn_b[ho]
            sd_tile = consts.tile([128, KC, BC], F32)
            for hoc in range(KC):
                ps_sd = att_psum.tile([128, BC], F32, tag="attps")
                for hic in range(KC):
                    nc.tensor.matmul(
                        ps_sd,
                        lhsT=w_tile[:, hic, hoc * 128:(hoc + 1) * 128],
                        rhs=dec_tile[:, hic, :],
                        start=(hic == 0),
                        stop=(hic == KC - 1),
                    )
                # ACT (not DVE tensor_scalar): the TensorScalarPtr ISA struct
                # only carries one sync-wait slot and this op needs two.
                nc.scalar.activation(
                    sd_tile[:, hoc, :],
                    ps_sd,
                    mybir.ActivationFunctionType.Identity,
                    bias=b_tile[:, hoc, :],
                )

            # Both mask rows up-front on SWDGE, ahead of the cast traffic and
            # off the sync HWDGE queue (a copy between transposes would cost
            # an xbar mode transition there).
            maskbs = []
            for b in range(BC):
                mb = rowsmall_pool.tile([1, L], BF, tag=f"maskb{b}")
                nc.gpsimd.dma_start(mb, maskadd[b:b + 1, :])
                maskbs.append(mb)

            # ---- main loop --------------------------------------------------
            full = [(tch * TCH, TCH) for tch in range(NCHUNK)]
            head = full
            for b in range(BC):
                chunks = full
                logits = rowbig_pool.tile([1, L], F32, tag="logits")
                mchunk = rowbig_pool.tile([1, len(head)], F32, tag="mchunk")
                maskb = maskbs[b]
                for ci, (t0, tw) in enumerate(chunks):
                    encT = encT_pool.tile([128, EC, tw], BF, tag="encT")
                    # SWDGE cast-DMA DRAM->DRAM (f32 -> bf16 scratch), split
                    # across the SWDGE queues, then ONE DRAM->SBUF xbar
                    # transpose per chunk: [tw tok, 2048 e] lands as
                    # encT[e % 128, e // 128, t] = enc[t, e].
                    scratch = scratch_pool.tile([tw, E], BF, tag="scratch")
                    for ts in range(max(tw // 64, 1)):
                        nc.gpsimd.dma_start(
                            scratch[ts * 64:(ts + 1) * 64, :],
                            enc[b, t0 + ts * 64:t0 + (ts + 1) * 64, :],
                        )
                    nc.sync.dma_start(encT[:, :, :], scratch[:, :], transpose=True)

                    en_big = en_pool.tile([128, KC, tw], BF, tag="en_big")
                    for hc in range(KC):
                        ps_score = score_psum.tile([128, tw], F32, tag="ps_score")
                        for ec in range(EC):
                            nc.tensor.matmul(
                                ps_score,
                                lhsT=w_tile[:, KC + ec, hc * 128:(hc + 1) * 128],
                                rhs=encT[:, ec, :],
                                start=(ec == 0),
                                stop=(ec == EC - 1),
                            )
                        nc.scalar.activation(
                            en_big[:, hc, :],
                            ps_score,
                            mybir.ActivationFunctionType.Tanh,
                            bias=sd_tile[:, hc, b:b + 1],
                        )

                    ps_att = att_psum.tile([1, tw], F32, tag="attps")
                    for hc in range(KC):
                        nc.tensor.matmul(
                            ps_att,
                            lhsT=vw_tile[:, hc, :],
                            rhs=en_big[:, hc, :],
                            start=(hc == 0),
                            stop=False,
                        )
                    # += (mask-1)*1e10 as a K=1 rank-1 update: masked tokens
                    # drop to ~-1e10 with no elementwise mask op anywhere.
                    nc.tensor.matmul(
                        ps_att,
                        lhsT=ones1,
                        rhs=maskb[:, t0:t0 + tw],
                        start=False,
                        stop=True,
                    )
                    nc.vector.tensor_copy(logits[:, t0:t0 + tw], ps_att)
                    # per-chunk max, computed while the PE crunches on — the
                    # final softmax then only reduces a handful of values.
                    nc.vector.reduce_max(
                        mchunk[:, ci:ci + 1],
                        logits[:, t0:t0 + tw],
                        axis=mybir.AxisListType.X,
                    )

                # ---- softmax over L on a single partition row --------------
                mx = rowsmall_pool.tile([1, 1], F32, tag="mx")
                nc.vector.reduce_max(
                    mx, mchunk[:, 0:len(chunks)], axis=mybir.AxisListType.X
                )
                negmx = rowsmall_pool.tile([1, 1], F32, tag="negmx")
                nc.scalar.mul(negmx, mx, -1.0)
                exps = rowsmall_pool.tile([1, L], F32, tag="exps")
                sumx = rowsmall_pool.tile([1, 1], F32, tag="sumx")
                nc.scalar.activation(
                    exps,
                    logits,
                    mybir.ActivationFunctionType.Exp,
                    bias=negmx[:, :],
                    accum_out=sumx,
                )
                rcp = rowsmall_pool.tile([1, 1], F32, tag="rcp")
                nc.vector.reciprocal(rcp, sumx)
                orow = rowbig_pool.tile([1, L], F32, tag="orow")
                nc.vector.tensor_scalar_mul(orow, exps, rcp[:, :])
                nc.gpsimd.dma_start(out[b:b + 1, :], orow)

    nc.finalize()
    return nc


_NC_CACHE = None


def _get_nc():
    global _NC_CACHE
    if _NC_CACHE is None:
        _NC_CACHE = build_nc()
    return _NC_CACHE


def prepare_in_maps(encoder_out, mask, v, attn_w, attn_b, v_w):
    encoder_out = np.ascontiguousarray(np.asarray(encoder_out, dtype=np.float32))
    maskadd = ((np.asarray(mask, dtype=np.float32) - 1.0) * 1.0e10).astype(BF16)
    wTb = np.ascontiguousarray(np.asarray(attn_w, dtype=np.float32).T).astype(BF16)
    decTb = np.ascontiguousarray(np.asarray(v[0], dtype=np.float32).T).astype(BF16)
    bcol = np.ascontiguousarray(np.asarray(attn_b, dtype=np.float32).reshape(H, 1))
    vwcol = np.ascontiguousarray(
        np.asarray(v_w, dtype=np.float32).reshape(H, 1)
    ).astype(BF16)

    in_maps = []
    for c in range(N_CORES):
        s = slice(c * BC, (c + 1) * BC)
        in_maps.append(
            {
                "encoder_out": encoder_out[s],
                "maskadd": maskadd[s],
                "attn_wT": wTb,
                "decT": np.ascontiguousarray(decTb[:, s]),
                "attn_bT": bcol,
                "v_wT": vwcol,
            }
        )
    return in_maps


def run(inputs, trace=False):
    nc = _get_nc()
    in_maps = prepare_in_maps(**inputs)
    res = run_bass_kernel_spmd(nc, in_maps, core_ids=list(range(N_CORES)), trace=trace)
    out = np.concatenate([res.results[c]["out"] for c in range(N_CORES)], axis=0)
    return out.astype(np.float32), res


def kernel(**inputs):
    out, _ = run(inputs, trace=False)
    return out
